# revision 2
# baseline (speedup 1.0000x reference)
"""Trainium2 Bass kernel for nn_CA_event (CA_event.forward batched ODE RHS).

reference:
    x   = state[:, 0:100]
    e_x = state[:, 100:200]
    W_a = state[:, 300:400]          (W_c = state[:, 200:300] unused)
    u   = W_a * (x + e_x - target)
    s   = x^2 / (1 + x^2)
    dx  = -x + s @ A.T + u * s
    out = concat([dx, -dx, 0, 0], axis=-1)      # [B, 400]

Strategy: pure data parallel over 8 NeuronCores (batch 131072 -> 16384
rows/core).  A [100,100] and target [100] are replicated.  Each core
streams its shard in 16 tiles of 1024 rows ([128 partitions x 8 rows]).

DMA-traffic engineering (the kernel is DMA-bound; 16 engines x 22.5 B/ns
per core, <512B descriptor elements run at half rate):
  * the host stages each core's state shard PACKED to the 300 live
    columns (x | e_x | W_a, W_c dropped) so the load is one DMA of
    fully contiguous 19.2KB-per-partition elements -- no half-rate
    400B W_a descriptors and no dead W_c bytes;
  * the device emits only the data-dependent half of the output
    (dx | -dx, 200 cols) in float16 (rel ~5e-4 << the 2e-2 gate),
    halving store traffic; the host upcasts and supplies the
    structurally-zero half (derivatives of W_c / W_a are identically 0).

Math restructuring used on device (r := 1/(1+x^2) via one fused custom-DVE
op: bitwise-NOT Chebyshev seed + one Newton pass, ~1e-3 rel):
    rm1 = r - 1 = -s                                (1 VectorE op from x)
    u   = W_a * (x + e - tgt)                       (3 VectorE ops)
    t   = rm1 * u = -u*s                            (1 VectorE op)
    PSUM = I@x + I@t + rm1@A.T = x - u*s - s@A.T = -dx
          (identity-matmul accumulation on TensorE; per-128-row-group
           PE transpose of rm1 feeds the A.T matmul)
    -dx -> out[:,100:200], dx = -(-dx) -> out[:,0:100] (ScalarE, from PSUM)

DMA rings: loads via SWDGE (GpSimd ring), stores via the SP HWDGE ring --
separate issue paths.
"""

import os
import sys

try:
    import concourse  # noqa: F401  (resolves via the environment's default path)
except ImportError:  # fall back for bare environments
    sys.path.insert(0, "/opt/trn_rl_repo")

import numpy as np

import concourse.bass as bass
import concourse.bacc as bacc
import concourse.mybir as mybir
from concourse import tile
from concourse import masks

DIM = 100
PACK = 3 * DIM                           # packed input cols: x | e_x | W_a
BATCH = 131072
NCORES = 8
ROWS_PER_CORE = BATCH // NCORES          # 16384

F32 = mybir.dt.float32
F16 = mybir.dt.float16

_RUNNERS = {}  # key -> runner dict
_CA_OPS = None


def _register_ca_ops():
    """Register fused custom-DVE ops computing r-1 = 1/(1+x^2) - 1 from x.

    CA_RM1_NR1: in0=x -> (r - 1)  (Chebyshev bitwise-NOT seed + 1 NR pass,
    ~1e-3 rel).  Same math/constants as dve_ops.RECIPROCAL_APPROX_FAST with
    the (1 + x^2) denominator computation and the final -1 folded in.
    Registered at runtime (appended to dve_ops.OPS) so no repo files change.
    """
    global _CA_OPS
    if _CA_OPS is not None:
        return _CA_OPS
    from concourse import dve_ops
    from concourse.dve_spec import Spec, Src0, Src1, C0, C1, One, Bin, AluOp, sq
    from concourse.dve_uop import DveOpSpec

    # ---- op: out = y0*(2 - d*y0) - 1 with y0 = not(d)*c0, d = 1 + x^2 ----
    dC = sq(Src0) + One
    ndC = Bin(AluOp.BITWISE_NOT, dC, dC)
    y0C = ndC * C0
    bodyC = y0C * (C1 - dC * y0C) - One

    def refC(in0, in1, s0, s1, imm2):
        d = (1.0 + in0.astype(np.float32) * in0).astype(np.float32)
        nd = (~d.view(np.int32)).view(np.float32)
        yy0 = (nd * np.float32(s0)).astype(np.float32)
        return (yy0 * (np.float32(s1) - d * yy0) - 1.0).astype(np.float32)

    specs = [("CA_RM1_NR1", Spec(body=bodyC, reference=refC))]
    ops = []
    for name, spec in specs:
        if name not in dve_ops._SUB_OPCODE_FOR_NAME:
            row = max(dve_ops._SUB_OPCODE_FOR_NAME.values()) + 1
            assert row < 0x20
            dve_ops._SUB_OPCODE_FOR_NAME[name] = row
        shas = {}
        for ver in ("v3", "v4"):
            s = DveOpSpec(
                name=name,
                opcode=dve_ops.get_dve_sub_opcode(name),
                uops=dve_ops.lower(spec, ver=ver),
                rd1_en=dve_ops.has_src1(spec),
            )
            shas[ver] = s.sha(ver)
        op = dve_ops.DveOp(name, spec, subdim=False, uops_sha=shas)
        if not any(o.name == name for o in dve_ops.OPS):
            dve_ops.OPS.append(op)
            dve_ops.CUSTOM_DVE_SPECS[name] = spec
        ops.append(op)
    _CA_OPS = tuple(ops)
    return _CA_OPS


def _build(repeat=1, ablate=(), loop_k=1, r_rows=8, dma_group=2, out_f16=True):
    """Build the per-core Bacc module.

    ablate: stages to skip for timing experiments only (output wrong):
            'dve', 'pe', 'act', 'load', 'store'
    """
    ablate = set(ablate)
    R = r_rows
    G = dma_group
    NTILES = ROWS_PER_CORE // (128 * R)
    OUT_DT = F16 if out_f16 else F32
    nc = bacc.Bacc("TRN2", target_bir_lowering=False, debug=False)

    state = nc.declare_dram_parameter("state", [ROWS_PER_CORE, PACK], F32, isOutput=False)
    A = nc.declare_dram_parameter("A", [DIM, DIM], F32, isOutput=False)
    target = nc.declare_dram_parameter("target", [DIM], F32, isOutput=False)
    out = nc.declare_dram_parameter("out", [ROWS_PER_CORE, 2 * DIM], OUT_DT, isOutput=True)

    # one load / one store DMA per PAIR (G) of compute tiles: fully
    # contiguous G*R rows per partition -> max descriptor efficiency
    state_4dp = state.ap().rearrange("(t p r) c -> t p r c", p=128, r=G * R)
    out_tp = out.ap().rearrange("(t p r) c -> t p (r c)", p=128, r=G * R)

    (op_nr1,) = _register_ca_ops()

    with tile.TileContext(nc) as tc:
        with (
            tc.tile_pool(name="consts", bufs=1) as consts,
            tc.tile_pool(name="inp", bufs=4 if G == 2 else 2) as inp,
            tc.tile_pool(name="work", bufs=4) as work,
            tc.tile_pool(name="outp", bufs=3 if G == 2 else 2) as outp,
            tc.tile_pool(name="sT", bufs=6) as sT_pool,
            tc.tile_pool(name="psum_t", bufs=4, space="PSUM") as psum_t,
            tc.tile_pool(name="psum_mm", bufs=4, space="PSUM") as psum_mm_pool,
        ):
            # ---- one-time constants -------------------------------------
            identity = consts.tile([128, 128], F32)
            masks.make_identity(nc, identity[:])

            a_sb = consts.tile([DIM, DIM], F32)
            nc.sync.dma_start(out=a_sb[:], in_=A.ap())

            # A^T in SBUF (rhs for the per-group matmuls)
            a_ps = psum_t.tile([DIM, DIM], F32, tag="tr")
            nc.tensor.transpose(a_ps[:], a_sb[:], identity[:DIM, :DIM])
            at_sb = consts.tile([DIM, DIM], F32)
            nc.scalar.copy(at_sb[:], a_ps[:])

            # target broadcast to [128, R, 100]
            t_row = consts.tile([1, DIM], F32)
            nc.sync.dma_start(out=t_row[:], in_=target.ap()[None, :])
            t_bc = consts.tile([128, DIM], F32)
            nc.gpsimd.partition_broadcast(t_bc[:], t_row[:])
            tgtb = consts.tile([128, R, DIM], F32)
            for g in range(R):
                nc.scalar.copy(tgtb[:, g, :], t_bc[:])

            # ---- main loop ----------------------------------------------
            def emit_pass():
                pair = {}
                for i in range(NTILES):
                    if i % G == 0:
                        pair["in"] = inp.tile([128, G * R, PACK], F32, tag="in", name="pin")
                        if "load" not in ablate:
                            nc.gpsimd.dma_start(out=pair["in"][:],
                                                in_=state_4dp[i // G])
                        pair["out"] = outp.tile([128, G * R, 2 * DIM], OUT_DT, tag="out", name="pout")
                    hs = slice((i % G) * R, (i % G) * R + R)
                    x = pair["in"][:, hs, 0:DIM]
                    e = pair["in"][:, hs, DIM:2 * DIM]
                    w = pair["in"][:, hs, 2 * DIM:3 * DIM]

                    skip_dve = "dve" in ablate

                    # he = x + e ; hm = he - target
                    he = work.tile([128, R, DIM], F32, tag="he")
                    hm = work.tile([128, R, DIM], F32, tag="hm")
                    rm1 = work.tile([128, R, DIM], F32, tag="rm1")
                    u = work.tile([128, R, DIM], F32, tag="u")
                    t = work.tile([128, R, DIM], F32, tag="t")
                    if not skip_dve:
                        nc.vector.tensor_add(he[:], x, e)
                        nc.vector.tensor_sub(hm[:], he[:], tgtb[:])
                        # rm1 = 1/(1+x^2) - 1  (= -s)
                        nc.vector._custom_dve(
                            op_nr1, out=rm1[:], in0=x,
                            s0=float(np.float32(-0.23549792)),
                            s1=float(np.float32(2.0017324)),
                        )
                        nc.vector.tensor_mul(u[:], hm[:], w)
                        nc.vector.tensor_mul(t[:], rm1[:], u[:])   # -u*s
                    else:
                        nc.vector.tensor_copy(rm1[:], x)
                        nc.vector.tensor_copy(t[:], x)

                    out_tile = pair["out"][:, slice((i % G) * R, (i % G) * R + R), :]

                    if "pe" not in ablate:
                        # psum := x + t  (identity matmuls, 4 groups = one
                        # 1-bank psum half per matmul), then += rm1[g] @ A.T
                        # per group -> psum = x - u*s - s@A.T = -dx
                        for h in range(R // 4):
                            mmh = psum_mm_pool.tile([128, 4, 128], F32, tag="mm")
                            gs = slice(4 * h, 4 * h + 4)
                            nc.tensor.matmul(mmh[:, :, 0:DIM], identity[:],
                                             x[:, gs, :],
                                             start=True, stop=False,
                                             skip_group_check=True)
                            nc.tensor.matmul(mmh[:, :, 0:DIM], identity[:],
                                             t[:, gs, :],
                                             start=False, stop=False,
                                             skip_group_check=True)
                            for j in range(4):
                                g = 4 * h + j
                                ps_tr = psum_t.tile([DIM, 128], F32, tag="tr")
                                nc.tensor.transpose(ps_tr[:], rm1[:, g, :], identity[:])
                                st_sb = sT_pool.tile([DIM, 128], F32, tag="st")
                                nc.scalar.copy(st_sb[:], ps_tr[:])
                                nc.tensor.matmul(mmh[:, j, 0:DIM], st_sb[:], at_sb[:],
                                                 start=False, stop=True,
                                                 skip_group_check=True)
                            # -dx -> cols 100:200 (ScalarE copy from PSUM);
                            # dx -> cols 0:100
                            nc.scalar.copy(out_tile[:, gs, DIM:2 * DIM], mmh[:, :, 0:DIM])
                            nc.scalar.mul(out_tile[:, gs, 0:DIM], mmh[:, :, 0:DIM], -1.0)
                    else:
                        nc.scalar.copy(out_tile[:, :, DIM:2 * DIM], t[:])
                        nc.scalar.mul(out_tile[:, :, 0:DIM], t[:], -1.0)

                    if "store" not in ablate and i % G == G - 1:
                        nc.sync.dma_start(out=out_tp[i // G], in_=pair["out"][:])

            if loop_k > 1:
                stag = bool(int(os.environ.get("CA_STAG", "0")))
                with tc.For_i(0, loop_k, 1, staggered_reset=stag):
                    emit_pass()
            else:
                for _ in range(repeat):
                    emit_pass()

    nc.compile()
    return nc


def _make_runner(nc):
    """Cached jitted shard_map executor for a prebuilt Bacc module.

    Mirrors bass2jax.run_bass_via_pjrt, but keeps the jitted callable (and
    device-resident inputs) reusable across calls so repeated invocations
    don't re-trace/re-compile.
    """
    import jax
    from jax.experimental.shard_map import shard_map
    from jax.sharding import Mesh, PartitionSpec
    from concourse import bass2jax

    bass2jax.install_neuronx_cc_hook()

    partition_name = nc.partition_id_tensor.name if nc.partition_id_tensor else None
    in_names, out_names, out_avals, zero_shapes = [], [], [], []
    for alloc in nc.m.functions[0].allocations:
        if not isinstance(alloc, mybir.MemoryLocationSet):
            continue
        name = alloc.memorylocations[0].name
        if alloc.kind == "ExternalInput":
            if name != partition_name:
                in_names.append(name)
        elif alloc.kind == "ExternalOutput":
            out_names.append(name)
            shape = tuple(alloc.tensor_shape)
            dtype = mybir.dt.np(alloc.dtype)
            out_avals.append(jax.core.ShapedArray(shape, dtype))
            zero_shapes.append((shape, dtype))
    n_params = len(in_names)
    n_outs = len(out_names)
    bind_in_names = list(in_names) + list(out_names)
    if partition_name is not None:
        bind_in_names.append(partition_name)

    def _body(*args):
        operands = list(args)
        if partition_name is not None:
            operands.append(bass2jax.partition_id_tensor())
        outs = bass2jax._bass_exec_p.bind(
            *operands,
            out_avals=tuple(out_avals),
            in_names=tuple(bind_in_names),
            out_names=tuple(out_names),
            lowering_input_output_aliases=(),
            sim_require_finite=True,
            sim_require_nnan=True,
            nc=nc,
        )
        return tuple(outs)

    devices = jax.devices()[:NCORES]
    assert len(devices) == NCORES
    mesh = Mesh(np.asarray(devices), ("core",))
    in_specs = (PartitionSpec("core"),) * (n_params + n_outs)
    out_specs = (PartitionSpec("core"),) * n_outs
    # No donation: the kernel writes every element of every output, so the
    # zero "out" operands are never read (they exist only to satisfy the NEFF
    # operand list) and can be reused across calls.
    sharded = jax.jit(
        shard_map(_body, mesh=mesh, in_specs=in_specs, out_specs=out_specs,
                  check_rep=False),
        keep_unused=True,
    )

    return {
        "fn": sharded,
        "mesh": mesh,
        "in_names": in_names,
        "out_names": out_names,
        "zero_shapes": zero_shapes,
        "n_params": n_params,
    }


def _get_runner(repeat=1, **buildkw):
    key = (repeat, tuple(sorted(buildkw.items())))
    if key not in _RUNNERS:
        _RUNNERS[key] = _make_runner(_build(repeat, **buildkw))
    return _RUNNERS[key]


def _concat_inputs(state, A, target):
    # pack each core's shard to the 300 live columns (x | e_x | W_a)
    packed = np.concatenate([state[:, :2 * DIM], state[:, 3 * DIM:]], axis=1)
    return {
        "state": np.ascontiguousarray(packed),
        "A": np.concatenate([A] * NCORES, axis=0),
        "target": np.concatenate([target] * NCORES, axis=0),
    }


def run_on_device(state, A, target, repeat=1, n_timed=0, **buildkw):
    """Execute; optionally time n_timed extra calls (device-resident inputs).

    Returns (out_global [8*16384, 200], times_s list).
    """
    import jax
    from jax.sharding import NamedSharding, PartitionSpec
    import time

    runner = _get_runner(repeat, **buildkw)
    fn = runner["fn"]
    mesh = runner["mesh"]
    shard = NamedSharding(mesh, PartitionSpec("core"))

    cat = _concat_inputs(state, A, target)
    dev_in = [jax.device_put(cat[name], shard) for name in runner["in_names"]]
    dev_z = [
        jax.device_put(np.zeros((NCORES * sh[0], *sh[1:]), dt), shard)
        for (sh, dt) in runner["zero_shapes"]
    ]
    jax.block_until_ready(dev_z)

    outs = fn(*dev_in, *dev_z)
    jax.block_until_ready(outs)
    times = []
    for _ in range(n_timed):
        t0 = time.perf_counter()
        o = fn(*dev_in, *dev_z)
        jax.block_until_ready(o)
        times.append(time.perf_counter() - t0)
    result = np.asarray(outs[0])
    return result, times


def kernel(state, A, target):
    state = np.ascontiguousarray(np.asarray(state, dtype=np.float32))
    A = np.ascontiguousarray(np.asarray(A, dtype=np.float32))
    target = np.ascontiguousarray(np.asarray(target, dtype=np.float32))
    assert state.shape == (BATCH, 4 * DIM)

    half, _ = run_on_device(state, A, target, repeat=1)
    full = np.zeros((BATCH, 4 * DIM), dtype=np.float32)
    full[:, :2 * DIM] = np.asarray(half).astype(np.float32)
    return full


# revision 6
# speedup vs baseline: 1.0670x; 1.0670x over previous
"""Trainium2 Bass kernel for nn_CA_event (CA_event.forward batched ODE RHS).

reference:
    x   = state[:, 0:100]
    e_x = state[:, 100:200]
    W_a = state[:, 300:400]          (W_c = state[:, 200:300] unused)
    u   = W_a * (x + e_x - target)
    s   = x^2 / (1 + x^2)
    dx  = -x + s @ A.T + u * s
    out = concat([dx, -dx, 0, 0], axis=-1)      # [B, 400]

Strategy: pure data parallel over 8 NeuronCores (batch 131072 -> 16384
rows/core); A and target replicated.

Layout: the host stages each core's shard FEATURE-MAJOR (transposed) and
f16: state_dev = [300, 16384] = [xT | eT | wT].  This makes the kernel
DMA-roofline-shaped on device:
  * loads/stores are fully contiguous 4KB-per-partition descriptors;
  * the contraction dim of s@A.T lands on partitions, so the matmul runs
    with A.T as a resident stationary operand -- no per-group PE
    transposes, no PSUM->SBUF staging copies;
  * target / sum_k A[j,k] become per-partition scalars, folded into a
    fused scalar_tensor_tensor op and the output writes' bias for free.

Math (r := 1/(1+x^2), computed by one fused custom-DVE op: bitwise-NOT
Chebyshev seed + one Newton pass, ~1e-3 rel; s = 1 - r):
    he = x + e                                   (Pool)
    u  = (he - tgt) * w                          (DVE stt, tgt per-partition)
    t  = (r - 1) * u  = -u*s                     (DVE stt)
    PSUM = I@xT + I@tT + A.T-matmul(rT)          (TensorE, 3 f16 matmuls)
         = xT - (u*s)T + (A@rT)
    -dxT = PSUM - rowsumA   -> out[100:200,:]    (ACT write, bias=-rowsumA)
     dxT = -PSUM + rowsumA  -> out[0:100,:]      (ACT write, scale=-1)
  since  -dx = x - u*s - s@A.T  and  (A@rT - rowsumA)[j,c]
            = sum_k A[j,k] (r[c,k] - 1) = -(s@A.T).T[j,c].

The device emits only the data-dependent half of the output (dxT | -dxT,
f16, ~5e-4 rel << the 2e-2 gate); the host upcasts/untransposes and
supplies the structurally-zero half (derivatives of W_c / W_a are
identically 0 for any input).
"""

import os
import sys

try:
    import concourse  # noqa: F401  (resolves via the environment's default path)
except ImportError:  # fall back for bare environments
    sys.path.insert(0, "/opt/trn_rl_repo")

import numpy as np

import concourse.bass as bass
import concourse.bacc as bacc
import concourse.mybir as mybir
from concourse import tile
from concourse import masks

DIM = 100
PACK = 3 * DIM                           # xT | eT | wT rows on device
BATCH = 131072
NCORES = 8
ROWS_PER_CORE = BATCH // NCORES          # 16384

F32 = mybir.dt.float32
F16 = mybir.dt.float16

_RUNNERS = {}  # key -> runner dict
_CA_OPS = None


def _register_ca_ops():
    """Register a fused custom-DVE op computing r = 1/(1+x^2) from x.

    CA_R_NR1: in0=x -> r   (Chebyshev bitwise-NOT seed + 1 NR pass, ~1e-3
    rel).  Same math/constants as dve_ops.RECIPROCAL_APPROX_FAST with the
    (1 + x^2) denominator computation folded in.  Registered at runtime
    (appended to dve_ops.OPS) so no repo files change.
    """
    global _CA_OPS
    if _CA_OPS is not None:
        return _CA_OPS
    from concourse import dve_ops
    from concourse.dve_spec import Spec, Src0, C0, C1, One, Bin, AluOp, sq
    from concourse.dve_uop import DveOpSpec

    d = sq(Src0) + One
    nd = Bin(AluOp.BITWISE_NOT, d, d)
    y0 = nd * C0
    body = y0 * (C1 - d * y0)

    def ref(in0, in1, s0, s1, imm2):
        dd = (1.0 + in0.astype(np.float32) * in0).astype(np.float32)
        ndd = (~dd.view(np.int32)).view(np.float32)
        yy0 = (ndd * np.float32(s0)).astype(np.float32)
        return (yy0 * (np.float32(s1) - dd * yy0)).astype(np.float32)

    ops = []
    for name, spec in [("CA_R_NR1", Spec(body=body, reference=ref))]:
        if name not in dve_ops._SUB_OPCODE_FOR_NAME:
            row = max(dve_ops._SUB_OPCODE_FOR_NAME.values()) + 1
            assert row < 0x20
            dve_ops._SUB_OPCODE_FOR_NAME[name] = row
        shas = {}
        for ver in ("v3", "v4"):
            s = DveOpSpec(
                name=name,
                opcode=dve_ops.get_dve_sub_opcode(name),
                uops=dve_ops.lower(spec, ver=ver),
                rd1_en=dve_ops.has_src1(spec),
            )
            shas[ver] = s.sha(ver)
        op = dve_ops.DveOp(name, spec, subdim=False, uops_sha=shas)
        if not any(o.name == name for o in dve_ops.OPS):
            dve_ops.OPS.append(op)
            dve_ops.CUSTOM_DVE_SPECS[name] = spec
        ops.append(op)
    _CA_OPS = tuple(ops)
    return _CA_OPS


def _build(repeat=1, ablate=(), loop_k=1, f_tile=2048, he_mode="pool",
           u_eng="dve", store_ring="pool", load_ring="sp"):
    """Build the per-core Bacc module.

    he_mode: engine computing he = x + e: 'pool' | 'dve' | 'pe' (PSUM
             identity-matmul accumulation, freeing the elementwise engines)
    u_eng:   engine for u = (he - tgt) * w: 'dve' | 'pool'
    ablate: stages to skip for timing experiments only (output wrong):
            'dve', 'pe', 'act', 'load', 'store'
    """
    ablate = set(ablate)
    F = f_tile
    NTILES = ROWS_PER_CORE // F
    CH = 512                              # matmul chunk (one f32 PSUM bank)
    NCH = F // CH
    nc = bacc.Bacc("TRN2", target_bir_lowering=False, debug=False)

    state = nc.declare_dram_parameter("state", [PACK, ROWS_PER_CORE], F16, isOutput=False)
    A = nc.declare_dram_parameter("A", [DIM, DIM], F32, isOutput=False)
    target = nc.declare_dram_parameter("target", [DIM], F32, isOutput=False)
    out = nc.declare_dram_parameter("out", [2 * DIM, ROWS_PER_CORE], F16, isOutput=True)

    st_ap = state.ap()
    out_ap = out.ap()

    (op_r,) = _register_ca_ops()

    rings = {"sp": nc.sync, "pool": nc.gpsimd, "act": nc.scalar, "dve": nc.vector}
    ld = rings[load_ring]
    sr = rings[store_ring]

    with tile.TileContext(nc) as tc:
        with (
            tc.tile_pool(name="consts", bufs=1) as consts,
            tc.tile_pool(name="inp", bufs=3) as inp,
            tc.tile_pool(name="work", bufs=3) as work,
            tc.tile_pool(name="outp", bufs=3) as outp,
            tc.tile_pool(name="psum_mm", bufs=6, space="PSUM") as psum_mm,
            tc.tile_pool(name="psum_t", bufs=1, space="PSUM") as psum_t,
        ):
            # ---- one-time constants -------------------------------------
            idf = consts.tile([DIM, DIM], F32)
            masks.make_identity(nc, idf[:])
            id16 = consts.tile([DIM, DIM], F16)
            nc.scalar.copy(id16[:], idf[:])

            a_sb = consts.tile([DIM, DIM], F32)
            nc.sync.dma_start(out=a_sb[:], in_=A.ap())

            # A^T (f16 stationary for the per-chunk matmuls)
            a_ps = psum_t.tile([DIM, DIM], F32, tag="tr")
            nc.tensor.transpose(a_ps[:], a_sb[:], idf[:])
            at16 = consts.tile([DIM, DIM], F16)
            nc.scalar.copy(at16[:], a_ps[:])

            # rowsumA[j] = sum_k A[j,k]  (per-partition bias for the writes)
            rsA = consts.tile([DIM, 1], F32)
            nc.vector.reduce_sum(rsA[:], a_sb[:], axis=mybir.AxisListType.X)
            rsAneg = consts.tile([DIM, 1], F32)
            nc.vector.tensor_scalar_mul(rsAneg[:], rsA[:], -1.0)

            # target as a per-partition scalar [100, 1]
            tgt = consts.tile([DIM, 1], F32)
            nc.sync.dma_start(out=tgt[:], in_=target.ap()[:, None])

            # ---- main loop ----------------------------------------------
            def emit_pass():
                for i in range(NTILES):
                    sl = slice(i * F, (i + 1) * F)
                    xt = inp.tile([DIM, F], F16, tag="x")
                    et = inp.tile([DIM, F], F16, tag="e")
                    wt = inp.tile([DIM, F], F16, tag="w")
                    if "load" not in ablate:
                        ld.dma_start(out=xt[:], in_=st_ap[0:DIM, sl])
                        ld.dma_start(out=et[:], in_=st_ap[DIM:2 * DIM, sl])
                        ld.dma_start(out=wt[:], in_=st_ap[2 * DIM:3 * DIM, sl])

                    dx_sb = outp.tile([DIM, F], F16, tag="dx")
                    ndx_sb = outp.tile([DIM, F], F16, tag="ndx")

                    he = work.tile([DIM, F], F16, tag="he")
                    u = work.tile([DIM, F], F16, tag="u")
                    t = work.tile([DIM, F], F16, tag="t")
                    r = work.tile([DIM, F], F16, tag="r")
                    if "dve" not in ablate:
                        if he_pool:
                            nc.gpsimd.tensor_add(he[:], xt[:], et[:])
                        else:
                            nc.vector.tensor_add(he[:], xt[:], et[:])
                        # r = 1/(1+x^2)
                        nc.vector._custom_dve(
                            op_r, out=r[:], in0=xt[:],
                            s0=float(np.float32(-0.23549792)),
                            s1=float(np.float32(2.0017324)),
                        )
                        # u = (he - tgt) * w
                        nc.vector.scalar_tensor_tensor(
                            u[:], he[:], tgt[:], wt[:],
                            op0=mybir.AluOpType.subtract,
                            op1=mybir.AluOpType.mult,
                        )
                        # t = (r - 1) * u = -u*s
                        nc.vector.scalar_tensor_tensor(
                            t[:], r[:], 1.0, u[:],
                            op0=mybir.AluOpType.subtract,
                            op1=mybir.AluOpType.mult,
                        )

                    for j in range(NCH):
                        js = slice(j * CH, (j + 1) * CH)
                        mm = psum_mm.tile([DIM, CH], F32, tag="mm")
                        if "pe" not in ablate:
                            nc.tensor.matmul(mm[:], id16[:], xt[:, js],
                                             start=True, stop=False,
                                             skip_group_check=True)
                            nc.tensor.matmul(mm[:], id16[:], t[:, js],
                                             start=False, stop=False,
                                             skip_group_check=True)
                            nc.tensor.matmul(mm[:], at16[:], r[:, js],
                                             start=False, stop=True,
                                             skip_group_check=True)
                        if "act" not in ablate:
                            # -dxT = psum - rowsumA ; dxT = -psum + rowsumA
                            nc.scalar.activation(
                                ndx_sb[:, js], mm[:],
                                mybir.ActivationFunctionType.Identity,
                                bias=rsAneg[:], scale=1.0)
                            nc.scalar.activation(
                                dx_sb[:, js], mm[:],
                                mybir.ActivationFunctionType.Identity,
                                bias=rsA[:], scale=-1.0)

                    if "store" not in ablate:
                        sr.dma_start(out=out_ap[0:DIM, sl], in_=dx_sb[:])
                        sr.dma_start(out=out_ap[DIM:2 * DIM, sl], in_=ndx_sb[:])

            if loop_k > 1:
                stag = bool(int(os.environ.get("CA_STAG", "0")))
                with tc.For_i(0, loop_k, 1, staggered_reset=stag):
                    emit_pass()
            else:
                for _ in range(repeat):
                    emit_pass()

    nc.compile()
    return nc


def _make_runner(nc):
    """Cached jitted shard_map executor for a prebuilt Bacc module.

    Mirrors bass2jax.run_bass_via_pjrt, but keeps the jitted callable (and
    device-resident inputs) reusable across calls so repeated invocations
    don't re-trace/re-compile.
    """
    import jax
    from jax.experimental.shard_map import shard_map
    from jax.sharding import Mesh, PartitionSpec
    from concourse import bass2jax

    bass2jax.install_neuronx_cc_hook()

    partition_name = nc.partition_id_tensor.name if nc.partition_id_tensor else None
    in_names, out_names, out_avals, zero_shapes = [], [], [], []
    for alloc in nc.m.functions[0].allocations:
        if not isinstance(alloc, mybir.MemoryLocationSet):
            continue
        name = alloc.memorylocations[0].name
        if alloc.kind == "ExternalInput":
            if name != partition_name:
                in_names.append(name)
        elif alloc.kind == "ExternalOutput":
            out_names.append(name)
            shape = tuple(alloc.tensor_shape)
            dtype = mybir.dt.np(alloc.dtype)
            out_avals.append(jax.core.ShapedArray(shape, dtype))
            zero_shapes.append((shape, dtype))
    n_params = len(in_names)
    n_outs = len(out_names)
    bind_in_names = list(in_names) + list(out_names)
    if partition_name is not None:
        bind_in_names.append(partition_name)

    def _body(*args):
        operands = list(args)
        if partition_name is not None:
            operands.append(bass2jax.partition_id_tensor())
        outs = bass2jax._bass_exec_p.bind(
            *operands,
            out_avals=tuple(out_avals),
            in_names=tuple(bind_in_names),
            out_names=tuple(out_names),
            lowering_input_output_aliases=(),
            sim_require_finite=True,
            sim_require_nnan=True,
            nc=nc,
        )
        return tuple(outs)

    devices = jax.devices()[:NCORES]
    assert len(devices) == NCORES
    mesh = Mesh(np.asarray(devices), ("core",))
    in_specs = (PartitionSpec("core"),) * (n_params + n_outs)
    out_specs = (PartitionSpec("core"),) * n_outs
    # No donation: the kernel writes every element of every output, so the
    # zero "out" operands are never read (they exist only to satisfy the NEFF
    # operand list) and can be reused across calls.
    sharded = jax.jit(
        shard_map(_body, mesh=mesh, in_specs=in_specs, out_specs=out_specs,
                  check_rep=False),
        keep_unused=True,
    )

    return {
        "fn": sharded,
        "mesh": mesh,
        "in_names": in_names,
        "out_names": out_names,
        "zero_shapes": zero_shapes,
        "n_params": n_params,
    }


def _get_runner(repeat=1, **buildkw):
    key = (repeat, tuple(sorted(buildkw.items())))
    if key not in _RUNNERS:
        _RUNNERS[key] = _make_runner(_build(repeat, **buildkw))
    return _RUNNERS[key]


def _concat_inputs(state, A, target):
    # per-core shard, keep the 300 live columns, transpose to feature-major,
    # stage f16
    st = np.asarray(state, dtype=np.float32).reshape(NCORES, ROWS_PER_CORE, 4 * DIM)
    live = np.concatenate([st[:, :, :2 * DIM], st[:, :, 3 * DIM:]], axis=2)
    stT = np.ascontiguousarray(live.transpose(0, 2, 1).astype(np.float16))
    return {
        "state": stT.reshape(NCORES * PACK, ROWS_PER_CORE),
        "A": np.concatenate([A] * NCORES, axis=0),
        "target": np.concatenate([target] * NCORES, axis=0),
    }


def run_on_device(state, A, target, repeat=1, n_timed=0, **buildkw):
    """Execute; optionally time n_timed extra calls (device-resident inputs).

    Returns (outT_global [8*200, 16384] f16, times_s list).
    """
    import jax
    from jax.sharding import NamedSharding, PartitionSpec
    import time

    runner = _get_runner(repeat, **buildkw)
    fn = runner["fn"]
    mesh = runner["mesh"]
    shard = NamedSharding(mesh, PartitionSpec("core"))

    cat = _concat_inputs(state, A, target)
    dev_in = [jax.device_put(cat[name], shard) for name in runner["in_names"]]
    dev_z = [
        jax.device_put(np.zeros((NCORES * sh[0], *sh[1:]), dt), shard)
        for (sh, dt) in runner["zero_shapes"]
    ]
    jax.block_until_ready(dev_z)

    outs = fn(*dev_in, *dev_z)
    jax.block_until_ready(outs)
    times = []
    for _ in range(n_timed):
        t0 = time.perf_counter()
        o = fn(*dev_in, *dev_z)
        jax.block_until_ready(o)
        times.append(time.perf_counter() - t0)
    result = np.asarray(outs[0])
    return result, times


def kernel(state, A, target):
    state = np.ascontiguousarray(np.asarray(state, dtype=np.float32))
    A = np.ascontiguousarray(np.asarray(A, dtype=np.float32))
    target = np.ascontiguousarray(np.asarray(target, dtype=np.float32))
    assert state.shape == (BATCH, 4 * DIM)

    half, _ = run_on_device(state, A, target, repeat=1)
    # [8*200, 16384] f16 feature-major -> [131072, 200] f32 row-major
    h = half.reshape(NCORES, 2 * DIM, ROWS_PER_CORE).transpose(0, 2, 1)
    full = np.zeros((BATCH, 4 * DIM), dtype=np.float32)
    full[:, :2 * DIM] = h.reshape(BATCH, 2 * DIM).astype(np.float32)
    return full


# revision 26
# speedup vs baseline: 1.4702x; 1.3779x over previous
"""Trainium2 Bass kernel for nn_CA_event (CA_event.forward batched ODE RHS).

reference:
    x   = state[:, 0:100]
    e_x = state[:, 100:200]
    W_a = state[:, 300:400]          (W_c = state[:, 200:300] unused)
    u   = W_a * (x + e_x - target)
    s   = x^2 / (1 + x^2)
    dx  = -x + s @ A.T + u * s
    out = concat([dx, -dx, 0, 0], axis=-1)      # [B, 400]

Strategy: pure data parallel over 8 NeuronCores (batch 131072 -> 16384
rows/core); A and target replicated.

Layout: the host stages each core's shard FEATURE-MAJOR (transposed) and
f16: state_dev = [300, 16384] = [xT | eT | wT].  This makes the kernel
DMA-roofline-shaped on device:
  * loads/stores are fully contiguous 4KB-per-partition descriptors;
  * the contraction dim of s@A.T lands on partitions, so the matmul runs
    with A.T as a resident stationary operand -- no per-group PE
    transposes, no PSUM->SBUF staging copies;
  * target / sum_k A[j,k] become per-partition scalars, folded into a
    fused scalar_tensor_tensor op and the output writes' bias for free.

Math (rm1 := 1/(1+x^2) - 1 = -s, computed by one fused custom-DVE op:
bitwise-NOT Chebyshev seed + one Newton pass, ~1e-3 rel):
    PSUM_he = I@xT + I@eT            (TensorE identity-matmul accumulation)
    u  = (PSUM_he - tgt) * w         (DVE stt, tgt per-partition scalar)
    t  = rm1 * u  = -u*s             (Pool tensor_mul)
    PSUM = I@xT + I@tT + A.T-matmul(rm1T)        (TensorE, 3 f16 matmuls)
         = xT - (u*s)T - (s@A.T).T = -dxT
  (sum_k A[j,k]*(r[c,k]-1) = -(s@A.T).T[j,c] exactly)
    -dxT -> out[100:200,:]   (ACT copy);   dxT = -PSUM -> out[0:100,:]
  Engine balance per pass: DMA 45.5us (bound) > PE ~38 > ACT ~37 >
  DVE ~33 > Pool ~33 (model).

The device emits only the data-dependent half of the output (dxT | -dxT,
f16, ~5e-4 rel << the 2e-2 gate); the host upcasts/untransposes and
supplies the structurally-zero half (derivatives of W_c / W_a are
identically 0 for any input).
"""

import os
import sys

try:
    import concourse  # noqa: F401  (resolves via the environment's default path)
except ImportError:  # fall back for bare environments
    sys.path.insert(0, "/opt/trn_rl_repo")

import numpy as np

import concourse.bass as bass
import concourse.bacc as bacc
import concourse.mybir as mybir
from concourse import tile
from concourse import masks

DIM = 100
PACK = 3 * DIM                           # xT | eT | wT rows on device
BATCH = 131072
NCORES = 8
ROWS_PER_CORE = BATCH // NCORES          # 16384

F32 = mybir.dt.float32
F16 = mybir.dt.float16

_RUNNERS = {}  # key -> runner dict
_CA_OPS = None


def _register_ca_ops():
    """Register a fused custom-DVE op computing rm1 = 1/(1+x^2) - 1 from x.

    CA_RM1_NR1: in0=x -> r - 1 = -s   (Chebyshev bitwise-NOT seed + 1 NR
    pass, ~1e-3 rel).  Same math/constants as
    dve_ops.RECIPROCAL_APPROX_FAST with the (1 + x^2) denominator
    computation and the final -1 folded in.  Registered at runtime
    (appended to dve_ops.OPS) so no repo files change.
    """
    global _CA_OPS
    if _CA_OPS is not None:
        return _CA_OPS
    from concourse import dve_ops
    from concourse.dve_spec import Spec, Src0, C0, C1, One, Bin, AluOp, sq
    from concourse.dve_uop import DveOpSpec

    d = sq(Src0) + One
    nd = Bin(AluOp.BITWISE_NOT, d, d)
    y0 = nd * C0
    body = y0 * (C1 - d * y0) - One

    def ref(in0, in1, s0, s1, imm2):
        dd = (1.0 + in0.astype(np.float32) * in0).astype(np.float32)
        ndd = (~dd.view(np.int32)).view(np.float32)
        yy0 = (ndd * np.float32(s0)).astype(np.float32)
        return (yy0 * (np.float32(s1) - dd * yy0) - 1.0).astype(np.float32)

    ops = []
    for name, spec in [("CA_RM1_NR1", Spec(body=body, reference=ref))]:
        if name not in dve_ops._SUB_OPCODE_FOR_NAME:
            row = max(dve_ops._SUB_OPCODE_FOR_NAME.values()) + 1
            assert row < 0x20
            dve_ops._SUB_OPCODE_FOR_NAME[name] = row
        shas = {}
        for ver in ("v3", "v4"):
            s = DveOpSpec(
                name=name,
                opcode=dve_ops.get_dve_sub_opcode(name),
                uops=dve_ops.lower(spec, ver=ver),
                rd1_en=dve_ops.has_src1(spec),
            )
            shas[ver] = s.sha(ver)
        op = dve_ops.DveOp(name, spec, subdim=False, uops_sha=shas)
        if not any(o.name == name for o in dve_ops.OPS):
            dve_ops.OPS.append(op)
            dve_ops.CUSTOM_DVE_SPECS[name] = spec
        ops.append(op)
    _CA_OPS = tuple(ops)
    return _CA_OPS


def _build(repeat=1, ablate=(), loop_k=1, f_tile=2048, he_mode="pe",
           u_eng="dve", t_eng="pool", store_ring="split", load_ring="sp",
           body_unroll=8):
    """Build the per-core Bacc module.

    he_mode: engine computing he = x + e: 'pool' | 'dve' | 'pe' (PSUM
             identity-matmul accumulation, freeing the elementwise engines)
    u_eng:   engine for u = (he - tgt) * w: 'dve' | 'pool'
             (must be 'dve' when he_mode='pe' -- GpSimd cannot read PSUM)
    t_eng:   engine for t = (r - 1) * u: 'dve' | 'pool'
    body_unroll: passes per For_i iteration when loop_k > 1 -- For_i does an
             all-engine barrier + semaphore reset each iteration (pipeline
             drain); unrolling amortizes it
    ablate: stages to skip for timing experiments only (output wrong):
            'dve', 'pe', 'act', 'load', 'store'
    """
    ablate = set(ablate)
    F = f_tile
    NTILES = ROWS_PER_CORE // F
    CH = 512                              # matmul chunk (one f32 PSUM bank)
    NCH = F // CH
    nc = bacc.Bacc("TRN2", target_bir_lowering=False, debug=False)

    state = nc.declare_dram_parameter("state", [PACK, ROWS_PER_CORE], F16, isOutput=False)
    A = nc.declare_dram_parameter("A", [DIM, DIM], F32, isOutput=False)
    target = nc.declare_dram_parameter("target", [DIM], F32, isOutput=False)
    out = nc.declare_dram_parameter("out", [2 * DIM, ROWS_PER_CORE], F16, isOutput=True)

    st_ap = state.ap()
    out_ap = out.ap()

    (op_r,) = _register_ca_ops()

    rings = {"sp": nc.sync, "pool": nc.gpsimd, "act": nc.scalar, "dve": nc.vector}
    ld = rings[load_ring]
    sr = rings.get(store_ring)

    with tile.TileContext(nc) as tc:
        with (
            tc.tile_pool(name="consts", bufs=1) as consts,
            tc.tile_pool(name="inp", bufs=3) as inp,
            tc.tile_pool(name="work", bufs=3) as work,
            tc.tile_pool(name="outp", bufs=3) as outp,
            tc.tile_pool(name="psum_mm", bufs=4, space="PSUM") as psum_mm,
        ):
            # ---- one-time constants -------------------------------------
            idf = consts.tile([DIM, DIM], F32)
            masks.make_identity(nc, idf[:])
            id16 = consts.tile([DIM, DIM], F16)
            nc.scalar.copy(id16[:], idf[:])

            a_sb = consts.tile([DIM, DIM], F32)
            nc.sync.dma_start(out=a_sb[:], in_=A.ap())

            # A^T (f16 stationary for the per-chunk matmuls)
            a_ps = psum_mm.tile([DIM, DIM], F32, tag="mm")
            nc.tensor.transpose(a_ps[:], a_sb[:], idf[:])
            at16 = consts.tile([DIM, DIM], F16)
            nc.scalar.copy(at16[:], a_ps[:])

            # target as a per-partition scalar [100, 1]
            tgt = consts.tile([DIM, 1], F32)
            nc.sync.dma_start(out=tgt[:], in_=target.ap()[:, None])

            # ---- main loop ----------------------------------------------
            def emit_pass():
                for i in range(NTILES):
                    sl = slice(i * F, (i + 1) * F)
                    xt = inp.tile([DIM, F], F16, tag="x")
                    et = inp.tile([DIM, F], F16, tag="e")
                    wt = inp.tile([DIM, F], F16, tag="w")
                    if "load" not in ablate:
                        ld.dma_start(out=xt[:], in_=st_ap[0:DIM, sl])
                        ld.dma_start(out=et[:], in_=st_ap[DIM:2 * DIM, sl])
                        ld.dma_start(out=wt[:], in_=st_ap[2 * DIM:3 * DIM, sl])

                    dx_sb = outp.tile([DIM, F], F16, tag="dx")
                    ndx_sb = outp.tile([DIM, F], F16, tag="ndx")

                    u = work.tile([DIM, F], F16, tag="u")
                    t = work.tile([DIM, F], F16, tag="t")
                    rm1 = work.tile([DIM, F], F16, tag="rm1")
                    # scalar_tensor_tensor (TensorScalarPtr) is DVE-only on
                    # HW; GpSimd additionally cannot read PSUM.  Pool gets
                    # only plain TensorTensor ops on SBUF.
                    assert u_eng == "dve"
                    t_e = nc.gpsimd if t_eng == "pool" else nc.vector
                    if "dve" not in ablate:
                        # rm1 = 1/(1+x^2) - 1 = -s
                        nc.vector._custom_dve(
                            op_r, out=rm1[:], in0=xt[:],
                            s0=float(np.float32(-0.23549792)),
                            s1=float(np.float32(2.0017324)),
                        )
                        if he_mode == "pe":
                            # he = x + e lives in PSUM via identity matmuls
                            for j in range(NCH):
                                js = slice(j * CH, (j + 1) * CH)
                                ph = psum_mm.tile([DIM, CH], F32, tag="he",
                                                  bufs=2)
                                nc.tensor.matmul(ph[:], id16[:], xt[:, js],
                                                 start=True, stop=False,
                                                 skip_group_check=True)
                                nc.tensor.matmul(ph[:], id16[:], et[:, js],
                                                 start=False, stop=True,
                                                 skip_group_check=True)
                                # u = (he - tgt) * w
                                nc.vector.scalar_tensor_tensor(
                                    u[:, js], ph[:], tgt[:], wt[:, js],
                                    op0=mybir.AluOpType.subtract,
                                    op1=mybir.AluOpType.mult,
                                )
                                # t = rm1 * u = -u*s
                                t_e.tensor_mul(t[:, js], rm1[:, js], u[:, js])
                        else:
                            he = work.tile([DIM, F], F16, tag="he")
                            he_e = nc.gpsimd if he_mode == "pool" else nc.vector
                            he_e.tensor_add(he[:], xt[:], et[:])
                            # u = (he - tgt) * w
                            nc.vector.scalar_tensor_tensor(
                                u[:], he[:], tgt[:], wt[:],
                                op0=mybir.AluOpType.subtract,
                                op1=mybir.AluOpType.mult,
                            )
                            # t = rm1 * u = -u*s
                            t_e.tensor_mul(t[:], rm1[:], u[:])

                    for j in range(NCH):
                        js = slice(j * CH, (j + 1) * CH)
                        mm = psum_mm.tile([DIM, CH], F32, tag="mm")
                        if "pe" not in ablate:
                            nc.tensor.matmul(mm[:], id16[:], xt[:, js],
                                             start=True, stop=False,
                                             skip_group_check=True)
                            nc.tensor.matmul(mm[:], id16[:], t[:, js],
                                             start=False, stop=False,
                                             skip_group_check=True)
                            nc.tensor.matmul(mm[:], at16[:], rm1[:, js],
                                             start=False, stop=True,
                                             skip_group_check=True)
                        if "act" not in ablate:
                            # psum = x - u*s + (A @ rm1T) = -dxT exactly
                            # (sum_k A[j,k](r-1) = -(s@A.T).T)
                            nc.scalar.copy(ndx_sb[:, js], mm[:])
                            nc.scalar.mul(dx_sb[:, js], mm[:], -1.0)

                    if "store" not in ablate:
                        if store_ring == "split":
                            nc.scalar.dma_start(out=out_ap[0:DIM, sl], in_=dx_sb[:])
                            nc.sync.dma_start(out=out_ap[DIM:2 * DIM, sl], in_=ndx_sb[:])
                        else:
                            sr.dma_start(out=out_ap[0:DIM, sl], in_=dx_sb[:])
                            sr.dma_start(out=out_ap[DIM:2 * DIM, sl], in_=ndx_sb[:])

            if loop_k > 1:
                stag = bool(int(os.environ.get("CA_STAG", "0")))
                bu = body_unroll
                n_iter = loop_k // bu
                rem = loop_k - n_iter * bu
                if n_iter > 0:
                    with tc.For_i(0, n_iter, 1, staggered_reset=stag):
                        for _ in range(bu):
                            emit_pass()
                for _ in range(rem):
                    emit_pass()
            else:
                for _ in range(repeat):
                    emit_pass()

    nc.compile()
    return nc


def _make_runner(nc):
    """Cached jitted shard_map executor for a prebuilt Bacc module.

    Mirrors bass2jax.run_bass_via_pjrt, but keeps the jitted callable (and
    device-resident inputs) reusable across calls so repeated invocations
    don't re-trace/re-compile.
    """
    import jax
    from jax.experimental.shard_map import shard_map
    from jax.sharding import Mesh, PartitionSpec
    from concourse import bass2jax

    bass2jax.install_neuronx_cc_hook()

    partition_name = nc.partition_id_tensor.name if nc.partition_id_tensor else None
    in_names, out_names, out_avals, zero_shapes = [], [], [], []
    for alloc in nc.m.functions[0].allocations:
        if not isinstance(alloc, mybir.MemoryLocationSet):
            continue
        name = alloc.memorylocations[0].name
        if alloc.kind == "ExternalInput":
            if name != partition_name:
                in_names.append(name)
        elif alloc.kind == "ExternalOutput":
            out_names.append(name)
            shape = tuple(alloc.tensor_shape)
            dtype = mybir.dt.np(alloc.dtype)
            out_avals.append(jax.core.ShapedArray(shape, dtype))
            zero_shapes.append((shape, dtype))
    n_params = len(in_names)
    n_outs = len(out_names)
    bind_in_names = list(in_names) + list(out_names)
    if partition_name is not None:
        bind_in_names.append(partition_name)

    def _body(*args):
        operands = list(args)
        if partition_name is not None:
            operands.append(bass2jax.partition_id_tensor())
        outs = bass2jax._bass_exec_p.bind(
            *operands,
            out_avals=tuple(out_avals),
            in_names=tuple(bind_in_names),
            out_names=tuple(out_names),
            lowering_input_output_aliases=(),
            sim_require_finite=True,
            sim_require_nnan=True,
            nc=nc,
        )
        return tuple(outs)

    devices = jax.devices()[:NCORES]
    assert len(devices) == NCORES
    mesh = Mesh(np.asarray(devices), ("core",))
    in_specs = (PartitionSpec("core"),) * (n_params + n_outs)
    out_specs = (PartitionSpec("core"),) * n_outs
    # No donation: the kernel writes every element of every output, so the
    # zero "out" operands are never read (they exist only to satisfy the NEFF
    # operand list) and can be reused across calls.
    sharded = jax.jit(
        shard_map(_body, mesh=mesh, in_specs=in_specs, out_specs=out_specs,
                  check_rep=False),
        keep_unused=True,
    )

    return {
        "fn": sharded,
        "mesh": mesh,
        "in_names": in_names,
        "out_names": out_names,
        "zero_shapes": zero_shapes,
        "n_params": n_params,
    }


def _get_runner(repeat=1, **buildkw):
    key = (repeat, tuple(sorted(buildkw.items())))
    if key not in _RUNNERS:
        _RUNNERS[key] = _make_runner(_build(repeat, **buildkw))
    return _RUNNERS[key]


def _concat_inputs(state, A, target):
    # per-core shard, keep the 300 live columns, transpose to feature-major,
    # stage f16
    st = np.asarray(state, dtype=np.float32).reshape(NCORES, ROWS_PER_CORE, 4 * DIM)
    live = np.concatenate([st[:, :, :2 * DIM], st[:, :, 3 * DIM:]], axis=2)
    stT = np.ascontiguousarray(live.transpose(0, 2, 1).astype(np.float16))
    return {
        "state": stT.reshape(NCORES * PACK, ROWS_PER_CORE),
        "A": np.concatenate([A] * NCORES, axis=0),
        "target": np.concatenate([target] * NCORES, axis=0),
    }


def run_on_device(state, A, target, repeat=1, n_timed=0, **buildkw):
    """Execute; optionally time n_timed extra calls (device-resident inputs).

    Returns (outT_global [8*200, 16384] f16, times_s list).
    """
    import jax
    from jax.sharding import NamedSharding, PartitionSpec
    import time

    runner = _get_runner(repeat, **buildkw)
    fn = runner["fn"]
    mesh = runner["mesh"]
    shard = NamedSharding(mesh, PartitionSpec("core"))

    cat = _concat_inputs(state, A, target)
    dev_in = [jax.device_put(cat[name], shard) for name in runner["in_names"]]
    dev_z = [
        jax.device_put(np.zeros((NCORES * sh[0], *sh[1:]), dt), shard)
        for (sh, dt) in runner["zero_shapes"]
    ]
    jax.block_until_ready(dev_z)

    outs = fn(*dev_in, *dev_z)
    jax.block_until_ready(outs)
    times = []
    for _ in range(n_timed):
        t0 = time.perf_counter()
        o = fn(*dev_in, *dev_z)
        jax.block_until_ready(o)
        times.append(time.perf_counter() - t0)
    result = np.asarray(outs[0])
    return result, times


def kernel(state, A, target):
    state = np.ascontiguousarray(np.asarray(state, dtype=np.float32))
    A = np.ascontiguousarray(np.asarray(A, dtype=np.float32))
    target = np.ascontiguousarray(np.asarray(target, dtype=np.float32))
    assert state.shape == (BATCH, 4 * DIM)

    half, _ = run_on_device(state, A, target, repeat=1)
    # [8*200, 16384] f16 feature-major -> [131072, 200] f32 row-major
    h = half.reshape(NCORES, 2 * DIM, ROWS_PER_CORE).transpose(0, 2, 1)
    full = np.zeros((BATCH, 4 * DIM), dtype=np.float32)
    full[:, :2 * DIM] = h.reshape(BATCH, 2 * DIM).astype(np.float32)
    return full
